# revision 6
# baseline (speedup 1.0000x reference)
"""Trainium2 Bass kernel for nn_AttentionMechanism (batched attention with
per-sample queries), data-parallel across 8 NeuronCores.

Math (per batch row b):
    q = msgs @ Wq.T + bq                         [H]
    k_t = Wk @ tau_t + bk ; scores_t = q.k_t/32
    alpha = softmax(scores) ; out = sum_t alpha_t (Wv @ tau_t + bv)

Rewrite (exact up to softmax shift invariance):
    Wqk  = (Wq.T @ Wk)/sqrt(H)   [MSG, TAU]   (host precompute)
    bqk  = (bq @ Wk)/sqrt(H)     [TAU]        (host precompute)
    qk   = msgs @ Wqk + bqk                    [TAU] per sample
    scores_t = qk . tau_t          (the q.bk term is constant in t -> cancels)
    p_t  = exp(scores_t)           (scores are O(1), no max-subtraction needed)
    ctx  = sum_t p_t tau_t / sum_t p_t
    out  = ctx @ Wv.T + bv         (uses sum alpha = 1)

Implementation notes:
  - host casts the big operands to bf16 and pre-transposes msgsT / WvT, so
    the device does no setup transposes at all; qk is 10 matmuls + a cast.
  - traj streams in t-chunks; per chunk the scores pipeline is
    DVE: prod (one fused op, qk broadcast over t) -> fold1 -> fold2,
    scalar: per-t 256-wide accum-reduce + exp,
    DVE: diag(p) build (one op), PE: 16 ctx matmuls.
  - GPSIMD is kept idle: it shares an SBUF port with the DVE and its ops
    measurably slow concurrent DVE work.
"""

import math

import numpy as np

try:
    import ml_dtypes

    BF16_NP = ml_dtypes.bfloat16
except ImportError:  # pragma: no cover
    import jax.numpy as jnp

    BF16_NP = jnp.bfloat16

import concourse.bass as bass
import concourse.bacc as bacc
import concourse.tile as tile
from concourse import mybir
from concourse.bass_utils import run_bass_kernel_spmd
from concourse.masks import make_identity

F32 = mybir.dt.float32
BF16 = mybir.dt.bfloat16

B = 2048
T = 32
TAU = 1024
MSG = 512
HID = 1024
VDIM = 128
N_CORES = 8
B_LOCAL = B // N_CORES

Alu = mybir.AluOpType
Act = mybir.ActivationFunctionType


def build(b_local=B_LOCAL, t_chunk=8, chunk_bufs=5, n_warm_mm=24,
          reduce_scalar_mod=8, diag_engine="vector"):
    assert b_local % 128 == 0 and T % t_chunk == 0
    n_btiles = b_local // 128
    n_chunks = T // t_chunk
    chunk_free = t_chunk * TAU

    nc = bacc.Bacc("TRN2", target_bir_lowering=False, debug=False)

    traj = nc.declare_dram_parameter(
        "imagined_trajectory", [b_local, T * TAU], BF16, isOutput=False
    )
    # host-packed: [p, j, ...] with p the partition dim, so each DMA is one
    # contiguous row per partition (trivial descriptors)
    msgsT = nc.declare_dram_parameter(
        "msgsT", [128, (MSG // 128) * b_local], BF16, isOutput=False
    )
    Wqk = nc.declare_dram_parameter(
        "Wqk", [128, (MSG // 128) * TAU], BF16, isOutput=False
    )
    bqk = nc.declare_dram_parameter("bqk", [1, TAU], BF16, isOutput=False)
    WvT = nc.declare_dram_parameter(
        "WvT", [128, (TAU // 128) * VDIM], BF16, isOutput=False
    )
    bv = nc.declare_dram_parameter("bv", [VDIM], F32, isOutput=False)
    out = nc.declare_dram_parameter("out", [b_local, VDIM], F32, isOutput=True)

    MQ = MSG // 128  # 4 m-chunks
    CQ = TAU // 128  # 8 c-chunks

    with tile.TileContext(nc) as tc:
        with (
            tc.tile_pool(name="const", bufs=1) as const,
            tc.tile_pool(name="persist", bufs=1) as persist,
            tc.tile_pool(name="stream", bufs=chunk_bufs) as stream,
            tc.tile_pool(name="work", bufs=1) as work,
            tc.tile_pool(name="foldp", bufs=2) as foldp,
            tc.tile_pool(name="diagp", bufs=2) as diagp,
            tc.tile_pool(name="aux", bufs=2) as aux,
            tc.tile_pool(name="outp", bufs=2) as outp,
            tc.tile_pool(name="psum_tr", bufs=2, space="PSUM") as psum_tr,
        ):
            # ---- DMA plan: traj chunks own the sync ring from t=0; all the
            # small setup operands ride the scalar ring in parallel.
            msgsT_sb = persist.tile([128, MQ, b_local], BF16)  # [m-part, j, b]
            nc.scalar.dma_start(
                out=msgsT_sb,
                in_=msgsT[:, :].rearrange("p (j b) -> p j b", j=MQ),
            )
            Wqk_sb = persist.tile([128, MQ, TAU], BF16)  # [m-part, j, c]
            nc.scalar.dma_start(
                out=Wqk_sb,
                in_=Wqk[:, :].rearrange("p (j c) -> p j c", j=MQ),
            )
            bqk_sb = const.tile([1, TAU], BF16)
            nc.scalar.dma_start(out=bqk_sb, in_=bqk[:, :])
            WvT_sb = persist.tile([128, CQ, VDIM], BF16)  # [c-part, j, d]
            nc.scalar.dma_start(
                out=WvT_sb,
                in_=WvT[:, :].rearrange("p (j d) -> p j d", j=CQ),
            )
            bv_sb = const.tile([1, VDIM], F32)
            nc.scalar.dma_start(out=bv_sb, in_=bv[None, :])

            # ---- constants ----
            ident_f = const.tile([128, 128], F32)
            make_identity(nc, ident_f)
            ident_b = const.tile([128, 128], BF16)
            make_identity(nc, ident_b)
            onespad_b = const.tile([128, 128], BF16)
            nc.vector.memset(onespad_b, 0.0)
            nc.vector.memset(onespad_b[0:1, :], 1.0)
            bvpad_b = const.tile([128, VDIM], BF16)
            nc.vector.memset(bvpad_b, 0.0)
            nc.vector.tensor_copy(out=bvpad_b[0:1, :], in_=bv_sb)
            ones_row = const.tile([1, 128], BF16)
            nc.vector.memset(ones_row, 1.0)

            # warm the PE (HAM) while the DMAs run
            for w in range(n_warm_mm):
                pw = psum_tr.tile([128, 128], F32, tag="tr", name="pw")
                nc.tensor.matmul(pw, lhsT=ident_b, rhs=ident_b, start=True, stop=True)

            qk_b = [
                persist.tile([128, TAU], BF16, tag=f"qkb{i}", name=f"qkb{i}")
                for i in range(n_btiles)
            ]

            # ---------- setup: qk = msgs @ Wqk + bqk (PE), then cast bf16
            with tc.tile_pool(name="psum_setup", bufs=2, space="PSUM") as psum_setup:
                for bi in range(n_btiles):
                    qp = psum_setup.tile(
                        [128, TAU], F32, tag="qkps", name=f"qp{bi}"
                    )
                    for nh in range(2):
                        nsl = slice(nh * 512, (nh + 1) * 512)
                        for j in range(MQ):
                            nc.tensor.matmul(
                                qp[:, nsl],
                                lhsT=msgsT_sb[:, j, bi * 128 : (bi + 1) * 128],
                                rhs=Wqk_sb[:, j, nsl],
                                start=(j == 0),
                                stop=False,
                            )
                        nc.tensor.matmul(
                            qp[:, nsl],
                            lhsT=ones_row,
                            rhs=bqk_sb[:, nsl],
                            start=False,
                            stop=True,
                        )
                    nc.vector.tensor_copy(out=qk_b[bi], in_=qp)

            # preload the exp table (after the setup DMA dispatches so the
            # ~2.7us table load does not block DMA issue on the scalar ring)
            exp_warm = const.tile([128, 1], F32)
            nc.vector.memset(exp_warm, 0.0)
            exp_warm2 = const.tile([128, 1], F32)
            nc.scalar.activation(out=exp_warm2, in_=exp_warm, func=Act.Exp)

            # ---------- main loop (flattened + software-pipelined) ----------
            # Stage A(u): dma chunk u, prod, fold1, fold2, reduces, exp
            # Stage B(u): diag(u), fillers, 16 ctx matmuls   (emitted one
            #   iteration later, so diag's wait on the scalar reduce chain
            #   never blocks the next chunk's DVE work in the queue)
            # Tail(bi): emitted right after B(last chunk of bi).
            psum_ctx_cm = tc.tile_pool(name="psum_ctx", bufs=2, space="PSUM")
            psum_ctx = psum_ctx_cm.__enter__()
            dump256 = aux.tile([128, 256], BF16, tag="dump256", name="dump256", bufs=1)
            n_units = n_btiles * n_chunks
            ident_rep = bass.AP(
                tensor=ident_b.tensor,
                offset=ident_b.offset,
                ap=[ident_b.ap[0], [0, t_chunk], ident_b.ap[1]],
            )
            state = {}  # per-unit tiles carried from stage A to stage B

            def stage_a(u):
                bi, ci = divmod(u, n_chunks)
                bsl = slice(bi * 128, (bi + 1) * 128)
                if ci == 0:
                    state[f"ctx{bi}"] = psum_ctx.tile(
                        [128, TAU], F32, tag="ctxps", name=f"ctx_ps{bi}"
                    )
                    state[f"scores{bi}"] = aux.tile(
                        [128, T], F32, tag="scores", name=f"scores{bi}"
                    )
                    state[f"p{bi}"] = aux.tile(
                        [128, T], BF16, tag="p", name=f"p{bi}"
                    )
                scores = state[f"scores{bi}"]
                p_b = state[f"p{bi}"]
                chunk_bf = stream.tile(
                    [128, t_chunk, TAU], BF16, tag="chunk", name="chunk_bf"
                )
                c0 = ci * chunk_free
                nc.sync.dma_start(
                    out=chunk_bf,
                    in_=traj[bsl, c0 : c0 + chunk_free].rearrange(
                        "p (t c) -> p t c", t=t_chunk
                    ),
                )
                # prod[:, t, c] = chunk[:, t, c] * qk[:, c] (one fused op,
                # qk broadcast over t via a 0-stride AP dim)
                prod = work.tile(
                    [128, t_chunk, TAU], BF16, tag="prod", name="prod"
                )
                qk_src = qk_b[bi]
                qk_rep = bass.AP(
                    tensor=qk_src.tensor,
                    offset=qk_src.offset,
                    ap=[qk_src.ap[0], [0, t_chunk], qk_src.ap[1]],
                )
                nc.vector.tensor_tensor(
                    out=prod, in0=chunk_bf, in1=qk_rep, op=Alu.mult
                )
                fold1 = work.tile(
                    [128, t_chunk, 512], BF16, tag="fold1", name="fold1"
                )
                nc.vector.tensor_tensor(
                    out=fold1,
                    in0=prod[:, :, 0:512],
                    in1=prod[:, :, 512:1024],
                    op=Alu.add,
                )
                fold2 = foldp.tile(
                    [128, t_chunk, 256], BF16, tag="fold2", name="fold2"
                )
                nc.vector.tensor_tensor(
                    out=fold2,
                    in0=fold1[:, :, 0:256],
                    in1=fold1[:, :, 256:512],
                    op=Alu.add,
                )
                terminal = u == n_units - 1
                for tt in range(t_chunk):
                    col = ci * t_chunk + tt
                    if not terminal and col % 8 < reduce_scalar_mod:
                        nc.scalar.activation(
                            out=dump256,
                            in_=fold2[:, tt, :],
                            func=Act.Copy,
                            accum_out=scores[:, col : col + 1],
                        )
                    else:
                        nc.vector.tensor_reduce(
                            out=scores[:, col : col + 1],
                            in_=fold2[:, tt, :],
                            axis=mybir.AxisListType.X,
                            op=Alu.add,
                        )
                csl = slice(ci * t_chunk, (ci + 1) * t_chunk)
                nc.scalar.activation(
                    out=p_b[:, csl], in_=scores[:, csl], func=Act.Exp
                )
                state[u] = (chunk_bf, prod, fold2)

            def stage_b(u):
                bi, ci = divmod(u, n_chunks)
                chunk_bf, prod, fold2 = state.pop(u)
                p_b = state[f"p{bi}"]
                ctx_ps = state[f"ctx{bi}"]
                csl = slice(ci * t_chunk, (ci + 1) * t_chunk)
                diag_c = diagp.tile(
                    [128, t_chunk, 128], BF16, tag="diag", name="diag_c"
                )
                p_sl = p_b[:, csl]
                p_rep = bass.AP(
                    tensor=p_sl.tensor,
                    offset=p_sl.offset,
                    ap=[p_sl.ap[0], [1, t_chunk], [0, 128]],
                )
                if diag_engine == "gpsimd":
                    nc.gpsimd.tensor_tensor(
                        out=diag_c, in0=ident_rep, in1=p_rep, op=Alu.mult
                    )
                else:
                    nc.vector.tensor_tensor(
                        out=diag_c, in0=ident_rep, in1=p_rep, op=Alu.mult
                    )
                # HAM fillers: cheap matmuls runnable mid-gap so the PE never
                # idles a full MID window and re-throttles.
                pw1 = psum_tr.tile([128, 128], F32, tag="tr", name="pw1")
                nc.tensor.matmul(
                    pw1, lhsT=ident_b, rhs=prod[:, 0, 0:128],
                    start=True, stop=True,
                )
                pw2 = psum_tr.tile([128, 128], F32, tag="tr", name="pw2")
                nc.tensor.matmul(
                    pw2, lhsT=ident_b, rhs=fold2[:, 0, 0:128],
                    start=True, stop=True,
                )
                for tt in range(t_chunk):
                    first = ci == 0 and tt == 0
                    last = ci == n_chunks - 1 and tt == t_chunk - 1
                    for nh in range(2):
                        nc.tensor.matmul(
                            ctx_ps[:, nh * 512 : (nh + 1) * 512],
                            lhsT=diag_c[:, tt, :],
                            rhs=chunk_bf[:, tt, nh * 512 : (nh + 1) * 512],
                            start=first,
                            stop=last,
                        )

            def tail(bi):
                bsl = slice(bi * 128, (bi + 1) * 128)
                p_b = state[f"p{bi}"]
                ctx_ps = state[f"ctx{bi}"]
                # normalize, project: out = (ctx / sum p) @ Wv.T + bv
                s_sum = aux.tile([128, 1], F32, tag="ssum", name="s_sum")
                nc.vector.tensor_reduce(
                    out=s_sum, in_=p_b, axis=mybir.AxisListType.X, op=Alu.add
                )
                rinv = aux.tile([128, 1], F32, tag="rinv", name="rinv")
                nc.vector.reciprocal(out=rinv, in_=s_sum)
                ctxn_f = aux.tile([128, TAU], F32, tag="ctxn", name="ctxn_f")
                nc.scalar.activation(
                    out=ctxn_f,
                    in_=ctx_ps,
                    func=Act.Copy,
                    scale=rinv,
                )
                ctxT_b = aux.tile([128, CQ, 128], BF16, tag="ctxT", name="ctxT_b")
                for j in range(CQ):
                    ptb = psum_tr.tile([128, 128], F32, tag="tr", name="ptb")
                    nc.tensor.transpose(
                        ptb, ctxn_f[:, j * 128 : (j + 1) * 128], ident_f
                    )
                    nc.scalar.copy(out=ctxT_b[:, j, :], in_=ptb)
                pm = psum_tr.tile([128, VDIM], F32, tag="tr", name="pm")
                for j in range(CQ):
                    nc.tensor.matmul(
                        pm,
                        lhsT=ctxT_b[:, j, :],
                        rhs=WvT_sb[:, j, :],
                        start=(j == 0),
                        stop=False,
                    )
                nc.tensor.matmul(
                    pm,
                    lhsT=onespad_b,
                    rhs=bvpad_b,
                    start=False,
                    stop=True,
                )
                msg_out = outp.tile([128, VDIM], F32, tag="msg", name="msg_out")
                nc.scalar.copy(out=msg_out, in_=pm)
                nc.scalar.dma_start(out=out[bsl, :], in_=msg_out)

            for u in range(n_units + 1):
                if u < n_units:
                    stage_a(u)
                if u >= 1:
                    stage_b(u - 1)
                    bi_prev, ci_prev = divmod(u - 1, n_chunks)
                    if ci_prev == n_chunks - 1:
                        tail(bi_prev)
            psum_ctx_cm.__exit__(None, None, None)

    nc.compile()
    return nc


_NC_CACHE = {}


def _get_nc():
    key = "default"
    if key not in _NC_CACHE:
        _NC_CACHE[key] = build()
    return _NC_CACHE[key]


def make_in_maps(imagined_trajectory, received_messages, Wq, bq, Wk, Wv, bv):
    bl = B_LOCAL
    scale = 1.0 / math.sqrt(HID)
    Wq32 = np.asarray(Wq, np.float32)
    Wk32 = np.asarray(Wk, np.float32)
    bq32 = np.asarray(bq, np.float32)
    # pack [K, N] (K = j*128 rows) as [128, j*N]: row p holds blocks j in order
    def pack_pj(a):  # a: [J*128, N] -> [128, J*N]
        J = a.shape[0] // 128
        return np.ascontiguousarray(
            a.reshape(J, 128, a.shape[1]).transpose(1, 0, 2).reshape(128, -1)
        )

    Wqk_h = pack_pj(((Wq32.T @ Wk32) * scale).astype(BF16_NP))
    bqk_h = ((bq32 @ Wk32) * scale)[None, :].astype(BF16_NP)
    WvT_h = pack_pj(
        np.ascontiguousarray(np.asarray(Wv, np.float32).T).astype(BF16_NP)
    )
    msgsT_h = np.ascontiguousarray(
        np.asarray(received_messages, np.float32).T
    ).astype(BF16_NP)
    traj_h = np.asarray(imagined_trajectory, np.float32).astype(BF16_NP)
    bv_h = np.asarray(bv, dtype=np.float32)
    in_maps = []
    for i in range(N_CORES):
        sl = slice(i * bl, (i + 1) * bl)
        in_maps.append(
            {
                "imagined_trajectory": np.ascontiguousarray(traj_h[sl]),
                "msgsT": pack_pj(np.ascontiguousarray(msgsT_h[:, sl])),
                "Wqk": Wqk_h,
                "bqk": bqk_h,
                "WvT": WvT_h,
                "bv": bv_h,
            }
        )
    return in_maps


def kernel(
    imagined_trajectory,
    received_messages,
    Wq,
    bq,
    Wk,
    bk,
    Wv,
    bv,
):
    nc = _get_nc()
    in_maps = make_in_maps(
        imagined_trajectory, received_messages, Wq, bq, Wk, Wv, bv
    )
    res = run_bass_kernel_spmd(nc, in_maps, list(range(N_CORES)))
    return np.concatenate([res.results[i]["out"] for i in range(N_CORES)], axis=0)


# revision 10
# speedup vs baseline: 1.1786x; 1.1786x over previous
"""Trainium2 Bass kernel for nn_AttentionMechanism (batched attention with
per-sample queries), data-parallel across 8 NeuronCores.

Math (per batch row b):
    q = msgs @ Wq.T + bq                         [H]
    k_t = Wk @ tau_t + bk ; scores_t = q.k_t/32
    alpha = softmax(scores) ; out = sum_t alpha_t (Wv @ tau_t + bv)

Rewrite (exact up to softmax shift invariance):
    Wqk  = (Wq.T @ Wk)/sqrt(H)   [MSG, TAU]   (host precompute)
    bqk  = (bq @ Wk)/sqrt(H)     [TAU]        (host precompute)
    qk   = msgs @ Wqk + bqk                    [TAU] per sample
    scores_t = qk . tau_t          (the q.bk term is constant in t -> cancels)
    p_t  = exp(scores_t)           (scores are O(1), no max-subtraction needed)
    ctx  = sum_t p_t tau_t / sum_t p_t
    out  = ctx @ Wv.T + bv         (uses sum alpha = 1)

Implementation notes:
  - host casts the big operands to bf16 and pre-transposes msgsT / WvT, so
    the device does no setup transposes at all; qk is 10 matmuls + a cast.
  - traj streams in t-chunks; per chunk the scores pipeline is
    DVE: prod (one fused op, qk broadcast over t) -> fold1 -> fold2,
    scalar: per-t 256-wide accum-reduce + exp,
    DVE: diag(p) build (one op), PE: 16 ctx matmuls.
  - GPSIMD is kept idle: it shares an SBUF port with the DVE and its ops
    measurably slow concurrent DVE work.
"""

import math

import numpy as np

try:
    import ml_dtypes

    BF16_NP = ml_dtypes.bfloat16
except ImportError:  # pragma: no cover
    import jax.numpy as jnp

    BF16_NP = jnp.bfloat16

import concourse.bass as bass
import concourse.bacc as bacc
import concourse.tile as tile
from concourse import mybir
from concourse.bass_utils import run_bass_kernel_spmd
from concourse.masks import make_identity

F32 = mybir.dt.float32
BF16 = mybir.dt.bfloat16

B = 2048
T = 32
TAU = 1024
MSG = 512
HID = 1024
VDIM = 128
N_CORES = 8
B_LOCAL = B // N_CORES

Alu = mybir.AluOpType
Act = mybir.ActivationFunctionType


def build(b_local=B_LOCAL, schedule=None, chunk_bufs=6, n_warm_mm=24,
          reduce_scalar_mod=8, diag_engine="vector"):
    assert b_local % 128 == 0
    n_btiles = b_local // 128
    if schedule is None:
        # per-btile t-chunk sizes; small edges shrink pipeline fill/drain
        schedule = [[2, 4, 8, 8, 8, 2], [8, 8, 8, 4, 2, 2]]
    assert len(schedule) == n_btiles and all(sum(s) == T for s in schedule)
    # flat unit list: (bi, ci, t_off, t_sz)
    units = []
    for bi, chunks in enumerate(schedule):
        off = 0
        for ci, ts in enumerate(chunks):
            units.append((bi, ci, off, ts))
            off += ts

    nc = bacc.Bacc("TRN2", target_bir_lowering=False, debug=False)

    traj = nc.declare_dram_parameter(
        "imagined_trajectory", [b_local, T * TAU], BF16, isOutput=False
    )
    # host-packed: [p, j, ...] with p the partition dim, so each DMA is one
    # contiguous row per partition (trivial descriptors)
    msgsT = nc.declare_dram_parameter(
        "msgsT", [128, (MSG // 128) * b_local], BF16, isOutput=False
    )
    Wqk = nc.declare_dram_parameter(
        "Wqk", [128, (MSG // 128) * TAU], BF16, isOutput=False
    )
    bqk = nc.declare_dram_parameter("bqk", [1, TAU], BF16, isOutput=False)
    WvT = nc.declare_dram_parameter(
        "WvT", [128, (TAU // 128) * VDIM], BF16, isOutput=False
    )
    bv = nc.declare_dram_parameter("bv", [VDIM], F32, isOutput=False)
    out = nc.declare_dram_parameter("out", [b_local, VDIM], F32, isOutput=True)

    MQ = MSG // 128  # 4 m-chunks
    CQ = TAU // 128  # 8 c-chunks

    with tile.TileContext(nc) as tc:
        with (
            tc.tile_pool(name="const", bufs=1) as const,
            tc.tile_pool(name="persist", bufs=1) as persist,
            tc.tile_pool(name="stream", bufs=chunk_bufs) as stream,
            tc.tile_pool(name="work", bufs=1) as work,
            tc.tile_pool(name="foldp", bufs=2) as foldp,
            tc.tile_pool(name="diagp", bufs=2) as diagp,
            tc.tile_pool(name="aux", bufs=2) as aux,
            tc.tile_pool(name="outp", bufs=2) as outp,
            tc.tile_pool(name="psum_tr", bufs=2, space="PSUM") as psum_tr,
        ):
            # ---- DMA plan: ALL DMAs ride the sync ring, setup operands
            # first (1.5MB, ~4.5us) so qk is ready before chunk 0 lands;
            # splitting rings just makes the SDMA engines round-robin and
            # delays the setup weights behind the 2MB traj chunks.
            msgsT_sb = persist.tile([128, MQ, b_local], BF16)  # [m-part, j, b]
            nc.sync.dma_start(
                out=msgsT_sb,
                in_=msgsT[:, :].rearrange("p (j b) -> p j b", j=MQ),
            )
            Wqk_sb = persist.tile([128, MQ, TAU], BF16)  # [m-part, j, c]
            nc.sync.dma_start(
                out=Wqk_sb,
                in_=Wqk[:, :].rearrange("p (j c) -> p j c", j=MQ),
            )
            bqk_sb = const.tile([1, TAU], BF16)
            nc.sync.dma_start(out=bqk_sb, in_=bqk[:, :])
            WvT_sb = persist.tile([128, CQ, VDIM], BF16)  # [c-part, j, d]
            nc.sync.dma_start(
                out=WvT_sb,
                in_=WvT[:, :].rearrange("p (j d) -> p j d", j=CQ),
            )
            bv_sb = const.tile([1, VDIM], F32)
            nc.sync.dma_start(out=bv_sb, in_=bv[None, :])

            # ---- constants ----
            ident_f = const.tile([128, 128], F32)
            make_identity(nc, ident_f)
            ident_b = const.tile([128, 128], BF16)
            make_identity(nc, ident_b)
            onespad_b = const.tile([128, 128], BF16)
            nc.vector.memset(onespad_b, 0.0)
            nc.vector.memset(onespad_b[0:1, :], 1.0)
            bvpad_b = const.tile([128, VDIM], BF16)
            nc.vector.memset(bvpad_b, 0.0)
            nc.vector.tensor_copy(out=bvpad_b[0:1, :], in_=bv_sb)
            ones_row = const.tile([1, 128], BF16)
            nc.vector.memset(ones_row, 1.0)

            # warm the PE (HAM) while the DMAs run
            for w in range(n_warm_mm):
                pw = psum_tr.tile([128, 128], F32, tag="tr", name="pw")
                nc.tensor.matmul(pw, lhsT=ident_b, rhs=ident_b, start=True, stop=True)

            qk_b = [
                persist.tile([128, TAU], BF16, tag=f"qkb{i}", name=f"qkb{i}")
                for i in range(n_btiles)
            ]

            # ---------- setup: qk = msgs @ Wqk + bqk (PE), then cast bf16
            with tc.tile_pool(name="psum_setup", bufs=2, space="PSUM") as psum_setup:
                for bi in range(n_btiles):
                    qp = psum_setup.tile(
                        [128, TAU], F32, tag="qkps", name=f"qp{bi}"
                    )
                    for nh in range(2):
                        nsl = slice(nh * 512, (nh + 1) * 512)
                        for j in range(MQ):
                            nc.tensor.matmul(
                                qp[:, nsl],
                                lhsT=msgsT_sb[:, j, bi * 128 : (bi + 1) * 128],
                                rhs=Wqk_sb[:, j, nsl],
                                start=(j == 0),
                                stop=False,
                            )
                        nc.tensor.matmul(
                            qp[:, nsl],
                            lhsT=ones_row,
                            rhs=bqk_sb[:, nsl],
                            start=False,
                            stop=True,
                        )
                    nc.vector.tensor_copy(out=qk_b[bi], in_=qp)

            # preload the exp table (after the setup DMA dispatches so the
            # ~2.7us table load does not block DMA issue on the scalar ring)
            exp_warm = const.tile([128, 1], F32)
            nc.vector.memset(exp_warm, 0.0)
            exp_warm2 = const.tile([128, 1], F32)
            nc.scalar.activation(out=exp_warm2, in_=exp_warm, func=Act.Exp)

            # ---------- main loop (flattened + software-pipelined) ----------
            # Stage A(u): dma chunk u, prod, fold1, fold2, reduces, exp
            # Stage B(u): diag(u), fillers, per-t ctx matmuls  (emitted one
            #   iteration later, so diag's wait on the scalar reduce chain
            #   never blocks the next chunk's DVE work in the queue)
            # Tail(bi): emitted right after B(last chunk of bi); normalize
            #   happens at the very end on the tiny [128, VDIM] output
            #   (out = (ctxT@WvT + Z*bv) * 1/Z), so the tail never waits on
            #   the softmax normalizer.
            psum_ctx_cm = tc.tile_pool(name="psum_ctx", bufs=2, space="PSUM")
            psum_ctx = psum_ctx_cm.__enter__()
            dump256 = aux.tile([128, 256], BF16, tag="dump256", name="dump256", bufs=1)
            n_units = len(units)
            state = {}  # per-unit tiles carried from stage A to stage B

            def stage_a(u):
                bi, ci, t_off, t_sz = units[u]
                bsl = slice(bi * 128, (bi + 1) * 128)
                if ci == 0:
                    state[f"ctx{bi}"] = psum_ctx.tile(
                        [128, TAU], F32, tag="ctxps", name=f"ctx_ps{bi}"
                    )
                    state[f"scores{bi}"] = aux.tile(
                        [128, T], F32, tag="scores", name=f"scores{bi}"
                    )
                    state[f"p{bi}"] = aux.tile(
                        [128, T], BF16, tag="p", name=f"p{bi}"
                    )
                scores = state[f"scores{bi}"]
                p_b = state[f"p{bi}"]
                chunk_bf = stream.tile(
                    [128, t_sz, TAU], BF16, tag="chunk", name="chunk_bf"
                )
                c0 = t_off * TAU
                nc.sync.dma_start(
                    out=chunk_bf,
                    in_=traj[bsl, c0 : c0 + t_sz * TAU].rearrange(
                        "p (t c) -> p t c", t=t_sz
                    ),
                )
                # prod[:, t, c] = chunk[:, t, c] * qk[:, c] (one fused op,
                # qk broadcast over t via a 0-stride AP dim)
                prod = work.tile(
                    [128, t_sz, TAU], BF16, tag="prod", name="prod"
                )
                qk_src = qk_b[bi]
                qk_rep = bass.AP(
                    tensor=qk_src.tensor,
                    offset=qk_src.offset,
                    ap=[qk_src.ap[0], [0, t_sz], qk_src.ap[1]],
                )
                nc.vector.tensor_tensor(
                    out=prod, in0=chunk_bf, in1=qk_rep, op=Alu.mult
                )
                fold1 = work.tile(
                    [128, t_sz, 512], BF16, tag="fold1", name="fold1"
                )
                nc.vector.tensor_tensor(
                    out=fold1,
                    in0=prod[:, :, 0:512],
                    in1=prod[:, :, 512:1024],
                    op=Alu.add,
                )
                fold2 = foldp.tile(
                    [128, t_sz, 256], BF16, tag="fold2", name="fold2"
                )
                nc.vector.tensor_tensor(
                    out=fold2,
                    in0=fold1[:, :, 0:256],
                    in1=fold1[:, :, 256:512],
                    op=Alu.add,
                )
                # last unit of each btile: reduce on the DVE so the diag /
                # boundary never waits on the serial scalar accum chain
                terminal = t_off + t_sz == T
                for tt in range(t_sz):
                    col = t_off + tt
                    if not terminal and col % 8 < reduce_scalar_mod:
                        nc.scalar.activation(
                            out=dump256,
                            in_=fold2[:, tt, :],
                            func=Act.Copy,
                            accum_out=scores[:, col : col + 1],
                        )
                    else:
                        nc.vector.tensor_reduce(
                            out=scores[:, col : col + 1],
                            in_=fold2[:, tt, :],
                            axis=mybir.AxisListType.X,
                            op=Alu.add,
                        )
                csl = slice(t_off, t_off + t_sz)
                nc.scalar.activation(
                    out=p_b[:, csl], in_=scores[:, csl], func=Act.Exp
                )
                state[u] = (chunk_bf, prod, fold2)

            def stage_b(u):
                bi, ci, t_off, t_sz = units[u]
                chunk_bf, prod, fold2 = state.pop(u)
                p_b = state[f"p{bi}"]
                ctx_ps = state[f"ctx{bi}"]
                csl = slice(t_off, t_off + t_sz)
                diag_c = diagp.tile(
                    [128, t_sz, 128], BF16, tag="diag", name="diag_c"
                )
                p_sl = p_b[:, csl]
                ident_rep = bass.AP(
                    tensor=ident_b.tensor,
                    offset=ident_b.offset,
                    ap=[ident_b.ap[0], [0, t_sz], ident_b.ap[1]],
                )
                p_rep = bass.AP(
                    tensor=p_sl.tensor,
                    offset=p_sl.offset,
                    ap=[p_sl.ap[0], [1, t_sz], [0, 128]],
                )
                if diag_engine == "gpsimd":
                    nc.gpsimd.tensor_tensor(
                        out=diag_c, in0=ident_rep, in1=p_rep, op=Alu.mult
                    )
                else:
                    nc.vector.tensor_tensor(
                        out=diag_c, in0=ident_rep, in1=p_rep, op=Alu.mult
                    )
                # HAM fillers: cheap matmuls runnable mid-gap so the PE never
                # idles a full MID window and re-throttles.
                pw1 = psum_tr.tile([128, 128], F32, tag="tr", name="pw1")
                nc.tensor.matmul(
                    pw1, lhsT=ident_b, rhs=prod[:, 0, 0:128],
                    start=True, stop=True,
                )
                pw2 = psum_tr.tile([128, 128], F32, tag="tr", name="pw2")
                nc.tensor.matmul(
                    pw2, lhsT=ident_b, rhs=fold2[:, 0, 0:128],
                    start=True, stop=True,
                )
                for tt in range(t_sz):
                    first = t_off + tt == 0
                    last = t_off + tt == T - 1
                    for nh in range(2):
                        nc.tensor.matmul(
                            ctx_ps[:, nh * 512 : (nh + 1) * 512],
                            lhsT=diag_c[:, tt, :],
                            rhs=chunk_bf[:, tt, nh * 512 : (nh + 1) * 512],
                            start=first,
                            stop=last,
                        )

            def tail(bi):
                bsl = slice(bi * 128, (bi + 1) * 128)
                p_b = state[f"p{bi}"]
                ctx_ps = state[f"ctx{bi}"]
                # Z = sum_t p_t (per row); out = (ctxT @ WvT + Z*bv) / Z
                s_sum = aux.tile([128, 1], F32, tag="ssum", name="s_sum")
                nc.vector.tensor_reduce(
                    out=s_sum, in_=p_b, axis=mybir.AxisListType.X, op=Alu.add
                )
                rinv = aux.tile([128, 1], F32, tag="rinv", name="rinv")
                nc.vector.reciprocal(out=rinv, in_=s_sum)
                # ZT row: [1, 128] bf16 (transpose of s_sum via PE)
                zt_ps = psum_tr.tile([1, 128], F32, tag="tr", name="zt_ps")
                nc.tensor.transpose(zt_ps, s_sum, ident_f)
                zt_row = aux.tile([1, 128], BF16, tag="zt", name="zt_row")
                nc.scalar.copy(out=zt_row, in_=zt_ps)
                # raw ctx -> SBUF bf16 (no scale), then transpose blocks
                ctxc_b = aux.tile([128, TAU], BF16, tag="ctxc", name="ctxc_b")
                nc.scalar.copy(out=ctxc_b, in_=ctx_ps)
                ctxT_b = aux.tile([128, CQ, 128], BF16, tag="ctxT", name="ctxT_b")
                for j in range(CQ):
                    ptb = psum_tr.tile([128, 128], BF16, tag="trb", name="ptb")
                    nc.tensor.transpose(
                        ptb, ctxc_b[:, j * 128 : (j + 1) * 128], ident_b
                    )
                    nc.vector.tensor_copy(out=ctxT_b[:, j, :], in_=ptb)
                pm = psum_tr.tile([128, VDIM], F32, tag="tr", name="pm")
                for j in range(CQ):
                    nc.tensor.matmul(
                        pm,
                        lhsT=ctxT_b[:, j, :],
                        rhs=WvT_sb[:, j, :],
                        start=(j == 0),
                        stop=False,
                    )
                nc.tensor.matmul(
                    pm,
                    lhsT=zt_row,
                    rhs=bvpad_b[0:1, :],
                    start=False,
                    stop=True,
                )
                msg_out = outp.tile([128, VDIM], F32, tag="msg", name="msg_out")
                nc.scalar.activation(
                    out=msg_out, in_=pm, func=Act.Copy, scale=rinv
                )
                nc.scalar.dma_start(out=out[bsl, :], in_=msg_out)

            n_per = [len(s) for s in schedule]
            for u in range(n_units + 1):
                if u < n_units:
                    stage_a(u)
                if u >= 1:
                    stage_b(u - 1)
                    bi_prev, ci_prev = units[u - 1][0], units[u - 1][1]
                    if ci_prev == n_per[bi_prev] - 1:
                        tail(bi_prev)
            psum_ctx_cm.__exit__(None, None, None)

    nc.compile()
    return nc


_NC_CACHE = {}


def _get_nc():
    key = "default"
    if key not in _NC_CACHE:
        _NC_CACHE[key] = build()
    return _NC_CACHE[key]


def make_in_maps(imagined_trajectory, received_messages, Wq, bq, Wk, Wv, bv):
    bl = B_LOCAL
    scale = 1.0 / math.sqrt(HID)
    Wq32 = np.asarray(Wq, np.float32)
    Wk32 = np.asarray(Wk, np.float32)
    bq32 = np.asarray(bq, np.float32)
    # pack [K, N] (K = j*128 rows) as [128, j*N]: row p holds blocks j in order
    def pack_pj(a):  # a: [J*128, N] -> [128, J*N]
        J = a.shape[0] // 128
        return np.ascontiguousarray(
            a.reshape(J, 128, a.shape[1]).transpose(1, 0, 2).reshape(128, -1)
        )

    Wqk_h = pack_pj(((Wq32.T @ Wk32) * scale).astype(BF16_NP))
    bqk_h = ((bq32 @ Wk32) * scale)[None, :].astype(BF16_NP)
    WvT_h = pack_pj(
        np.ascontiguousarray(np.asarray(Wv, np.float32).T).astype(BF16_NP)
    )
    msgsT_h = np.ascontiguousarray(
        np.asarray(received_messages, np.float32).T
    ).astype(BF16_NP)
    traj_h = np.asarray(imagined_trajectory, np.float32).astype(BF16_NP)
    bv_h = np.asarray(bv, dtype=np.float32)
    in_maps = []
    for i in range(N_CORES):
        sl = slice(i * bl, (i + 1) * bl)
        in_maps.append(
            {
                "imagined_trajectory": np.ascontiguousarray(traj_h[sl]),
                "msgsT": pack_pj(np.ascontiguousarray(msgsT_h[:, sl])),
                "Wqk": Wqk_h,
                "bqk": bqk_h,
                "WvT": WvT_h,
                "bv": bv_h,
            }
        )
    return in_maps


def kernel(
    imagined_trajectory,
    received_messages,
    Wq,
    bq,
    Wk,
    bk,
    Wv,
    bv,
):
    nc = _get_nc()
    in_maps = make_in_maps(
        imagined_trajectory, received_messages, Wq, bq, Wk, Wv, bv
    )
    res = run_bass_kernel_spmd(nc, in_maps, list(range(N_CORES)))
    return np.concatenate([res.results[i]["out"] for i in range(N_CORES)], axis=0)
